# revision 9
# baseline (speedup 1.0000x reference)
"""Trainium2 Bass/Tile kernel for nn_CAVAModule (cross-attention A/V alignment).

Math notes (exact simplifications of the reference):
  - delta = 2 + 4*sigmoid(clip(theta,-12,12)) is in [2, 6], so the mask
    center min(max(t+delta,0),t) == t for every t: the displacement-aware
    causal mask is a fixed 6-tap causal moving average, independent of theta.
  - The soft temporal shift (2-tap linear interp at integer lag n=floor(delta))
    composed with that moving average is a banded Toeplitz operator over time
    with a <=8-tap band. Only rows t < 12 (clipping at t=0) deviate from the
    Toeplitz pattern, so the whole shift+mask+einsum collapses into three
    128x128 blocks: C00 (tile 0), Csub/Cdiag (every later tile), applied as
    PE matmuls against the 128-token LayerNorm'd audio tiles.

Sharding: data-parallel over batch, one sample per NeuronCore, no cross-core
communication. Each core runs the full per-sample pipeline over 16 tiles of
128 tokens.
"""

import sys

for _p in ("/opt/trn_rl_repo",):
    if _p not in sys.path:
        sys.path.insert(0, _p)

import numpy as np

import concourse.bacc as bacc
import concourse.bass as bass
import concourse.tile as tile
from concourse import mybir
from concourse.bass_utils import run_bass_kernel_spmd

F32 = mybir.dt.float32
F32R = mybir.dt.float32r
ALU = mybir.AluOpType
ACT = mybir.ActivationFunctionType

B, T, VDIM, ADIM, DM = 8, 2048, 1024, 768, 256
HID = 1024
P = 128
NT = T // P          # 16 token tiles
KV = VDIM // P       # 8
KA = ADIM // P       # 6
KX = (3 * DM) // P   # 6
CHUNK = 2            # token tiles per input DMA chunk
LN_EPS = 1e-5
L2_EPS = 1e-8
WIN = 6              # mask window taps (tau in [t-5, t])

_nc_cache: dict = {}


def _build_cmats(delta: float) -> np.ndarray:
    """Three [tau, t] blocks of the combined shift+mask operator."""
    dl = min(max(delta, 0.0), float(T - 1))
    n = int(np.floor(dl))
    alpha = dl - n

    def row_w(t):
        w = np.zeros(2 * P, np.float64)
        m = min(t + 1, WIN)
        for s in range(max(0, t - (WIN - 1)), t + 1):
            i0 = min(max(s - n, 0), T - 1)
            i1 = min(i0 + 1, T - 1)
            w[i0] += (1.0 - alpha) / m
            w[i1] += alpha / m
        return w

    c00 = np.zeros((P, P), np.float64)
    csub = np.zeros((P, P), np.float64)
    cdiag = np.zeros((P, P), np.float64)
    for t in range(P):
        w = row_w(t)
        c00[:, t] = w[:P]
        w = row_w(P + t)
        csub[:, t] = w[:P]
        cdiag[:, t] = w[P:2 * P]
    return np.ascontiguousarray(np.stack([c00, csub, cdiag]).astype(np.float32))


def _build(bv_nz: bool, ba_nz: bool, b1_nz: bool, b2f: float, upto: int = 99, logits_mode: str = 'stt'):
    nc = bacc.Bacc("TRN2", target_bir_lowering=False, debug=False, num_devices=8)

    # f32r (TF32-like PE fast path) for every tensor consumed only by matmuls;
    # numpy side is still plain float32 bits — the PE rounds on read.
    vT = nc.dram_tensor("vT", [VDIM, T], F32R, kind="ExternalInput")
    aT = nc.dram_tensor("aT", [ADIM, T], F32R, kind="ExternalInput")
    wv = nc.dram_tensor("wv", [VDIM, DM], F32R, kind="ExternalInput")
    wa = nc.dram_tensor("wa", [ADIM, DM], F32R, kind="ExternalInput")
    w1 = nc.dram_tensor("w1", [3 * DM, HID], F32R, kind="ExternalInput")
    w2 = nc.dram_tensor("w2", [HID], F32, kind="ExternalInput")
    cm = nc.dram_tensor("cm", [3, P, P], F32R, kind="ExternalInput")
    ident = nc.dram_tensor("ident", [P, P], F32, kind="ExternalInput")
    if bv_nz:
        bvr = nc.dram_tensor("bvr", [1, DM], F32R, kind="ExternalInput")
    if ba_nz:
        bar = nc.dram_tensor("bar", [1, DM], F32R, kind="ExternalInput")
    if b1_nz:
        b1r = nc.dram_tensor("b1r", [1, HID], F32R, kind="ExternalInput")
    out = nc.dram_tensor("out", [T, DM], F32, kind="ExternalOutput")

    def bcast(handle_ap, n):
        # partition-broadcast a [1, n] / [n] DRAM AP to 128 partitions
        return bass.AP(
            tensor=handle_ap.tensor, offset=handle_ap.offset, ap=[[0, P], [1, n]]
        )

    with tile.TileContext(nc) as tc:
        with (
            tc.tile_pool(name="singles", bufs=1) as singles,
            tc.tile_pool(name="vchunk", bufs=2) as vchunk,
            tc.tile_pool(name="achunk", bufs=2) as achunk,
            tc.tile_pool(name="va", bufs=3) as va_pool,
            tc.tile_pool(name="araw", bufs=2) as araw_pool,
            tc.tile_pool(name="scratch", bufs=2) as scratch,
            tc.tile_pool(name="xt", bufs=2) as xt_pool,
            tc.tile_pool(name="hbuf", bufs=2) as hbuf,
            tc.tile_pool(name="obuf", bufs=3) as obuf,
            tc.tile_pool(name="small", bufs=8) as small,
            tc.tile_pool(name="psum_mm", bufs=2, space="PSUM") as psum_mm,
            tc.tile_pool(name="psum_c", bufs=1, space="PSUM") as psum_c,
            tc.tile_pool(name="psum_t", bufs=1, space="PSUM") as psum_t,
            tc.tile_pool(name="psum_h", bufs=2, space="PSUM") as psum_h,
        ):
            # ---- persistent weights/constants ----
            wv_sb = singles.tile([P, KV, DM], F32R)
            nc.sync.dma_start(out=wv_sb, in_=wv.ap().rearrange("(ko p) n -> p ko n", p=P))
            wa_sb = singles.tile([P, KA, DM], F32R)
            nc.sync.dma_start(out=wa_sb, in_=wa.ap().rearrange("(ko p) n -> p ko n", p=P))
            w1_sb = singles.tile([P, KX, HID], F32R)
            nc.sync.dma_start(out=w1_sb, in_=w1.ap().rearrange("(ko p) n -> p ko n", p=P))
            w2_sb = singles.tile([P, HID], F32)
            nc.gpsimd.dma_start(out=w2_sb, in_=bcast(w2.ap(), HID))
            cm_sb = singles.tile([P, 3, P], F32R)
            nc.sync.dma_start(out=cm_sb, in_=cm.ap().rearrange("c p t -> p c t"))
            id_sb = singles.tile([P, P], F32)
            nc.sync.dma_start(out=id_sb, in_=ident.ap())
            eps_sb = singles.tile([P, 1], F32)
            nc.vector.memset(eps_sb, LN_EPS)
            if bv_nz or ba_nz or b1_nz:
                ones_sb = singles.tile([1, P], F32R)
                nc.vector.memset(ones_sb, 1.0)
            if bv_nz:
                bv_sb = singles.tile([1, DM], F32R)
                nc.sync.dma_start(out=bv_sb, in_=bvr.ap())
            if ba_nz:
                ba_sb = singles.tile([1, DM], F32R)
                nc.sync.dma_start(out=ba_sb, in_=bar.ap())
            if b1_nz:
                b1_sb = singles.tile([1, HID], F32R)
                nc.sync.dma_start(out=b1_sb, in_=b1r.ap())

            vT_r = vT.ap().rearrange("(ko p) t -> p ko t", p=P)
            aT_r = aT.ap().rearrange("(ko p) t -> p ko t", p=P)

            def layernorm(psum_in, dst_pool, dst_dtype=F32):
                """LN over free dim of [P, DM] PSUM tile -> SBUF tile."""
                stats = small.tile([P, nc.vector.BN_STATS_DIM], F32)
                nc.vector.bn_stats(out=stats, in_=psum_in)
                mv = small.tile([P, nc.vector.BN_AGGR_DIM], F32)
                nc.vector.bn_aggr(out=mv, in_=stats)
                std = small.tile([P, 1], F32)
                nc.scalar.activation(out=std, in_=mv[:, 1:2], func=ACT.Sqrt,
                                     bias=eps_sb, scale=1.0)
                rstd = small.tile([P, 1], F32)
                nc.vector.reciprocal(out=rstd, in_=std)
                nmur = small.tile([P, 1], F32)
                nc.vector.tensor_scalar(out=nmur, in0=mv[:, 0:1], scalar1=rstd,
                                        scalar2=-1.0, op0=ALU.mult, op1=ALU.mult)
                dst = dst_pool.tile([P, DM], dst_dtype)
                nc.scalar.activation(out=dst, in_=psum_in, func=ACT.Identity,
                                     bias=nmur, scale=rstd)
                return dst

            def l2scale(src_sb_or_psum, dst):
                """dst = src / max(||src||_2(row), eps); returns nothing."""
                sq = scratch.tile([P, DM], F32, tag="sq")
                ssq = small.tile([P, 1], F32)
                nc.scalar.activation(out=sq, in_=src_sb_or_psum, func=ACT.Square,
                                     accum_out=ssq)
                nrm = small.tile([P, 1], F32)
                nc.scalar.activation(out=nrm, in_=ssq, func=ACT.Sqrt)
                rn = small.tile([P, 1], F32)
                nc.vector.tensor_scalar_max(out=nrm, in0=nrm, scalar1=L2_EPS)
                nc.vector.reciprocal(out=rn, in_=nrm)
                nc.scalar.activation(out=dst, in_=src_sb_or_psum, func=ACT.Copy,
                                     scale=rn)

            a_prev = None
            for i in range(NT):
                c, sel = divmod(i, CHUNK)
                if sel == 0:
                    vt_sb = vchunk.tile([P, KV, CHUNK * P], F32R)
                    nc.sync.dma_start(
                        out=vt_sb, in_=vT_r[:, :, c * CHUNK * P:(c + 1) * CHUNK * P])
                    at_sb = achunk.tile([P, KA, CHUNK * P], F32R)
                    nc.sync.dma_start(
                        out=at_sb, in_=aT_r[:, :, c * CHUNK * P:(c + 1) * CHUNK * P])
                tsl = slice(sel * P, (sel + 1) * P)

                # ---- v = LN(video @ Wv + bv) ----
                pv = psum_mm.tile([P, DM], F32, tag="mm")
                for k in range(KV):
                    nc.tensor.matmul(pv, lhsT=vt_sb[:, k, tsl],
                                     rhs=wv_sb[:, k, :],
                                     start=(k == 0), stop=(k == KV - 1 and not bv_nz))
                if bv_nz:
                    nc.tensor.matmul(pv, lhsT=ones_sb, rhs=bv_sb,
                                     start=False, stop=True)
                v_sb = layernorm(pv, va_pool)

                # ---- a = LN(audio @ Wa + ba) ----
                pa = psum_mm.tile([P, DM], F32, tag="mm")
                for k in range(KA):
                    nc.tensor.matmul(pa, lhsT=at_sb[:, k, tsl],
                                     rhs=wa_sb[:, k, :],
                                     start=(k == 0), stop=(k == KA - 1 and not ba_nz))
                if ba_nz:
                    nc.tensor.matmul(pa, lhsT=ones_sb, rhs=ba_sb,
                                     start=False, stop=True)
                a_sb = layernorm(pa, va_pool, F32R)

                # ---- a_ctx = banded (shift + causal window mean) over time ----
                pc = psum_c.tile([P, DM], F32)
                if i == 0:
                    nc.tensor.matmul(pc, lhsT=cm_sb[:, 0, :], rhs=a_sb,
                                     start=True, stop=True)
                else:
                    nc.tensor.matmul(pc, lhsT=cm_sb[:, 1, :], rhs=a_prev,
                                     start=True, stop=False)
                    nc.tensor.matmul(pc, lhsT=cm_sb[:, 2, :], rhs=a_sb,
                                     start=False, stop=True)
                a_prev = a_sb
                actx_sb = araw_pool.tile([P, DM], F32)
                nc.scalar.activation(out=actx_sb, in_=pc, func=ACT.Copy)
                if upto <= 2:
                    nc.sync.dma_start(out=out.ap()[i * P:(i + 1) * P, :], in_=actx_sb)
                    continue

                # ---- l2 normalize, x = [an, vn, an*vn] transposed ----
                vn_sb = scratch.tile([P, DM], F32, tag="vn")
                l2scale(v_sb, vn_sb)
                an_sb = scratch.tile([P, DM], F32, tag="an")
                l2scale(actx_sb, an_sb)

                if upto <= 3:
                    nc.sync.dma_start(out=out.ap()[i * P:(i + 1) * P, :], in_=vn_sb)
                    continue
                pt = psum_t.tile([P, 4, P], F32)
                for k in range(2):
                    nc.tensor.transpose(pt[:, k, :], an_sb[:, k * P:(k + 1) * P], id_sb)
                for k in range(2):
                    nc.tensor.transpose(pt[:, 2 + k, :], vn_sb[:, k * P:(k + 1) * P], id_sb)
                xt = xt_pool.tile([P, KX, P], F32R)
                nc.scalar.copy(out=xt[:, 0:4, :], in_=pt)
                nc.vector.tensor_mul(out=xt[:, 4:6, :], in0=xt[:, 0:2, :],
                                     in1=xt[:, 2:4, :])

                # ---- h = gelu(x @ W1 + b1) ----
                ph = psum_h.tile([P, HID], F32)
                for nh in range(2):
                    psl = ph[:, nh * 512:(nh + 1) * 512]
                    for k in range(KX):
                        nc.tensor.matmul(psl, lhsT=xt[:, k, :],
                                         rhs=w1_sb[:, k, nh * 512:(nh + 1) * 512],
                                         start=(k == 0), stop=(k == KX - 1 and not b1_nz))
                    if b1_nz:
                        nc.tensor.matmul(psl, lhsT=ones_sb,
                                         rhs=b1_sb[:, nh * 512:(nh + 1) * 512],
                                         start=False, stop=True)
                h_sb = hbuf.tile([P, HID], F32, tag="h")
                nc.scalar.activation(out=h_sb[:, 0:512], in_=ph[:, 0:512], func=ACT.Gelu)
                nc.scalar.activation(out=h_sb[:, 512:HID], in_=ph[:, 512:HID], func=ACT.Gelu)
                if upto <= 4:
                    nc.sync.dma_start(out=out.ap()[i * P:(i + 1) * P, :], in_=h_sb[:, 0:DM])
                    continue

                # ---- logits = clip(h @ W2 + b2), g = clip(sigmoid) ----
                lsc = hbuf.tile([P, HID], F32, tag="lsc")
                lg = small.tile([P, 1], F32)
                if logits_mode == "ttr":
                    nc.vector.tensor_tensor_reduce(out=lsc, in0=h_sb, in1=w2_sb,
                                                   scale=1.0, scalar=float(b2f),
                                                   op0=ALU.mult, op1=ALU.add,
                                                   accum_out=lg)
                elif logits_mode == "stt":
                    nc.vector.scalar_tensor_tensor(out=lsc, in0=h_sb, scalar=1.0,
                                                   in1=w2_sb, op0=ALU.mult,
                                                   op1=ALU.mult, accum_out=lg)
                    if b2f != 0.0:
                        nc.vector.tensor_scalar_add(out=lg, in0=lg, scalar1=float(b2f))
                else:
                    nc.vector.tensor_mul(out=lsc, in0=h_sb, in1=w2_sb)
                    nc.vector.reduce_sum(out=lg, in_=lsc, axis=mybir.AxisListType.X)
                    if b2f != 0.0:
                        nc.vector.tensor_scalar_add(out=lg, in0=lg, scalar1=float(b2f))
                nc.vector.tensor_scalar(out=lg, in0=lg, scalar1=-12.0, scalar2=12.0,
                                        op0=ALU.max, op1=ALU.min)
                gg = small.tile([P, 1], F32)
                nc.scalar.activation(out=gg, in_=lg, func=ACT.Sigmoid)
                nc.vector.tensor_scalar(out=gg, in0=gg, scalar1=0.05, scalar2=0.95,
                                        op0=ALU.max, op1=ALU.min)
                if upto <= 5:
                    nc.sync.dma_start(out=out.ap()[i * P:(i + 1) * P, 0:1], in_=gg)
                    continue

                # ---- fused = g*a_ctx + (1-g)*v = (a_ctx - v)*g + v ----
                d_sb = scratch.tile([P, DM], F32, tag="d")
                nc.vector.tensor_tensor(out=d_sb, in0=actx_sb, in1=v_sb,
                                        op=ALU.subtract)
                o_sb = obuf.tile([P, DM], F32)
                nc.vector.scalar_tensor_tensor(out=o_sb, in0=d_sb, scalar=gg,
                                               in1=v_sb, op0=ALU.mult, op1=ALU.add)
                nc.sync.dma_start(out=out.ap()[i * P:(i + 1) * P, :], in_=o_sb)

    nc.compile()
    return nc


def kernel(video_seq, audio_seq, Wv, bv, Wa, ba, theta, W1, b1, W2, b2):
    video_seq = np.asarray(video_seq, np.float32)
    audio_seq = np.asarray(audio_seq, np.float32)
    th = float(np.clip(np.float32(theta), -12.0, 12.0))
    delta = 2.0 + 4.0 / (1.0 + np.exp(-th))
    cmats = _build_cmats(float(delta))

    bv_nz = bool(np.any(np.asarray(bv) != 0))
    ba_nz = bool(np.any(np.asarray(ba) != 0))
    b1_nz = bool(np.any(np.asarray(b1) != 0))
    b2f = float(np.asarray(b2).reshape(-1)[0])

    key = (bv_nz, ba_nz, b1_nz, b2f)
    if key not in _nc_cache:
        _nc_cache[key] = _build(bv_nz, ba_nz, b1_nz, b2f)
    nc = _nc_cache[key]

    shared = {
        "wv": np.ascontiguousarray(Wv, np.float32),
        "wa": np.ascontiguousarray(Wa, np.float32),
        "w1": np.ascontiguousarray(W1, np.float32),
        "w2": np.ascontiguousarray(np.asarray(W2, np.float32).reshape(HID)),
        "cm": cmats,
        "ident": np.eye(P, dtype=np.float32),
    }
    if bv_nz:
        shared["bvr"] = np.ascontiguousarray(np.asarray(bv, np.float32).reshape(1, DM))
    if ba_nz:
        shared["bar"] = np.ascontiguousarray(np.asarray(ba, np.float32).reshape(1, DM))
    if b1_nz:
        shared["b1r"] = np.ascontiguousarray(np.asarray(b1, np.float32).reshape(1, HID))

    in_maps = []
    for b in range(B):
        m = dict(shared)
        m["vT"] = np.ascontiguousarray(video_seq[b].T)
        m["aT"] = np.ascontiguousarray(audio_seq[b].T)
        in_maps.append(m)

    res = run_bass_kernel_spmd(nc, in_maps, list(range(B)))
    return np.stack([res.results[i]["out"] for i in range(B)])
